# revision 25
# baseline (speedup 1.0000x reference)
"""Trainium2 Bass kernel for BilinearResNet (batch-data-parallel over 8 cores).

kernel(**inputs) takes the FULL unsharded numpy inputs (keys as in
setup_inputs()) and returns the full output tuple (logits, h1..h4), matching
reference() in shapes/dtypes. Full fp32 precision (rel err ~5e-7).

Design (per core, batch shard of 8192 rows):
  - Host pre-transposes x so the kernel streams xT [784, B] with perfectly
    coalesced DMA; all on-chip activations are kept transposed
    [feature, batch]; outputs are written transposed and un-transposed on
    the host. No on-chip transposes anywhere.
  - Every matmul contracts over the partition dim. 4 batch groups of 512
    columns run concurrently on the PE via 4x column tiling
    (tile_position=(0, 32g)), which also packs the 4 groups into one PSUM
    bank so each DVE/ACT op serves 2048 batch columns.
  - The residual stream h_x accumulates in PSUM across embed (7 K-chunks)
    plus the 4 per-block D-matmuls as one accumulation group per group
    strip; u and v go to separate PSUM banks at the same partitions so
    u*v is a single partition-aligned DVE multiply per block.
  - Weights are host-packed into zero-padded, partition-replicated lhsT
    layouts so every matmul is K<=128 with no fixups; the leftover
    K=16 embed chunk rides in a zeroed full-height tile.
  - Queues: x loads on Sync (HWDGE), h outputs on Scalar (HWDGE), logits
    on Sync, xlast on GpSimd (SWDGE) - chain-critical PSUM evacuation
    stays on DVE so DMA-lane stalls never back up the compute chain.
"""

import os
import sys

for _p in ("/opt/trn_rl_repo", "/root/.axon_site/_ro/trn_rl_repo"):
    if os.path.isdir(_p) and _p not in sys.path:
        sys.path.append(_p)

import numpy as np

# Problem constants (hardcoded per harness contract).
IN_DIM, D_MODEL, HIDDEN, N_BLOCKS, N_CLASSES = 784, 16, 6, 4, 10
BATCH = 65536
N_CORES = 8
B_CORE = BATCH // N_CORES  # 8192
NSUP = 4                   # super-tiles per core
SUP = B_CORE // NSUP       # 2048 batch columns per super-tile
NT = 512                   # batch columns per PE column-tile group (PSUM bank)
NG = 4                     # concurrent column-tile groups
NCHUNK_FULL = IN_DIM // 128          # 6 full K chunks
K_LAST = IN_DIM - 128 * NCHUNK_FULL  # 16 leftover contraction rows

_CACHE = {}


def _build_nc(debug=False, mm_dt="f32"):
    import concourse.bacc as bacc
    import concourse.mybir as mybir
    import concourse.tile as tile

    f32 = mybir.dt.float32
    # "f32r": declare the whole fp32 dataflow as float32r so the PE streams
    # it single-pass (1 cycle/row) instead of fp32's 4; PSUM stays fp32.
    dio = mybir.dt.float32r if mm_dt == "f32r" else f32
    nc = bacc.Bacc(
        "TRN2",
        target_bir_lowering=False,
        debug=debug,
        num_devices=N_CORES,
    )

    xT = nc.dram_tensor("xT", [IN_DIM, B_CORE], dio, kind="ExternalInput").ap()
    wet = nc.dram_tensor("wet", [128, 7, 16], dio, kind="ExternalInput").ap()
    lw = nc.dram_tensor("lw", [128, N_BLOCKS, NG, 6], dio, kind="ExternalInput").ap()
    rw = nc.dram_tensor("rw", [128, N_BLOCKS, NG, 6], dio, kind="ExternalInput").ap()
    dwt = nc.dram_tensor("dwt", [128, N_BLOCKS, NG, 16], dio, kind="ExternalInput").ap()
    wht = nc.dram_tensor("wht", [128, NG, 10], dio, kind="ExternalInput").ap()
    lgT = nc.dram_tensor("lgT", [N_CLASSES, B_CORE], dio, kind="ExternalOutput").ap()
    hT = nc.dram_tensor("hT", [N_BLOCKS, HIDDEN, B_CORE], dio, kind="ExternalOutput").ap()
    # [r, i, c] view so one DMA per group can write all 4 blocks' h rows.
    hT_r = hT.rearrange("i r c -> r i c")

    with tile.TileContext(nc) as tc:
        from contextlib import ExitStack

        with ExitStack() as stack:
            wp = stack.enter_context(tc.tile_pool(name="weights", bufs=1))
            xp = stack.enter_context(tc.tile_pool(name="xin", bufs=1))
            hp = stack.enter_context(tc.tile_pool(name="hx", bufs=4))
            up = stack.enter_context(tc.tile_pool(name="uv", bufs=3))
            tp = stack.enter_context(tc.tile_pool(name="ht", bufs=2))
            lp = stack.enter_context(tc.tile_pool(name="lg", bufs=2))
            pp = stack.enter_context(tc.tile_pool(name="ps", bufs=1, space="PSUM"))

            wet_sb = wp.tile([128, 7, 16], dio, tag="wet")
            lw_sb = wp.tile([128, N_BLOCKS, NG, 6], dio, tag="lw")
            rw_sb = wp.tile([128, N_BLOCKS, NG, 6], dio, tag="rw")
            dwt_sb = wp.tile([128, N_BLOCKS, NG, 16], dio, tag="dwt")
            wht_sb = wp.tile([128, NG, 10], dio, tag="wht")
            weight_loads = [
                (wet_sb, wet), (lw_sb, lw), (rw_sb, rw), (dwt_sb, dwt), (wht_sb, wht)
            ]

            # Leftover K chunk (rows 768:784 of xT) lives in partitions 0:16 of
            # a full-height tile; partitions 16:128 are zeroed once so the
            # K=128 embed matmul against zero-padded weights reads finite data.
            xlast = xp.tile([128, B_CORE], dio, tag="xlast")
            nc.gpsimd.memset(xlast[:].bitcast(f32), 0.0)
            nc.gpsimd.dma_start(xlast[0:16, :], xT[768:784, :])

            xts = [
                xp.tile([128, NCHUNK_FULL, SUP], dio, tag=f"xts{j}", name=f"xts{j}")
                for j in (0, 1)
            ]
            ph = [
                pp.tile([128, NT], f32, tag=f"ph{j}", name=f"ph{j}") for j in (0, 1, 2)
            ]
            pu = [pp.tile([128, NT], f32, tag=f"pu{j}", name=f"pu{j}") for j in (0, 1)]
            pv = [pp.tile([128, NT], f32, tag=f"pv{j}", name=f"pv{j}") for j in (0, 1)]
            plg = [pp.tile([128, NT], f32, tag="plg0", name="plg0")]
            # One-time zero of u/v/logit banks: rows never written by their
            # matmuls stay finite (and zero) for the full-tile reads.
            for t in (*pu, *pv, *plg):
                nc.vector.memset(t[:], 0.0)

            for s in range(NSUP):
                sl = s % 2
                x_sb = xts[sl]
                if s == 0:
                    nc.sync.dma_start(weight_loads[0][0][:], weight_loads[0][1][:])
                for q in range(NCHUNK_FULL):
                    nc.sync.dma_start(
                        x_sb[:, q, :],
                        xT[128 * q : 128 * (q + 1), s * SUP : (s + 1) * SUP],
                    )
                if s == 0:
                    for dst, srcw in weight_loads[1:]:
                        nc.sync.dma_start(dst[:], srcw[:])
                p_h = ph[s % 3]
                for q in range(7):
                    for g in range(NG):
                        if q < NCHUNK_FULL:
                            rhs = x_sb[:, q, g * NT : (g + 1) * NT]
                        else:
                            c0 = s * SUP + g * NT
                            rhs = xlast[:, c0 : c0 + NT]
                        nc.tensor.matmul(
                            p_h[32 * g : 32 * g + 16, :],
                            wet_sb[:, q, :],
                            rhs,
                            start=(q == 0),
                            stop=False,
                            skip_group_check=True,
                            tile_position=(0, 32 * g),
                        )
                hx = hp.tile([128, NT], dio, tag="hx", name="hx")
                nc.vector.tensor_copy(hx[:], p_h[:])

                # h outputs for all 4 blocks stage here, then leave in one
                # DMA per group.
                ht_st = tp.tile([128, N_BLOCKS, NT], dio, tag="ht", name="ht")
                for i in range(N_BLOCKS):
                    p_u = pu[sl]
                    p_v = pv[sl]
                    for g in range(NG):
                        nc.tensor.matmul(
                            p_u[32 * g : 32 * g + 6, :],
                            lw_sb[:, i, g, :],
                            hx[:],
                            start=True,
                            stop=True,
                            tile_position=(0, 32 * g),
                        )
                    for g in range(NG):
                        nc.tensor.matmul(
                            p_v[32 * g : 32 * g + 6, :],
                            rw_sb[:, i, g, :],
                            hx[:],
                            start=True,
                            stop=True,
                            tile_position=(0, 32 * g),
                        )
                    uvs = up.tile([128, NT], dio, tag="uvs", name="uvs")
                    nc.vector.tensor_copy(uvs[:], p_u[:])
                    # h = u * v for all 4 groups at once; u and v live at the
                    # same partitions (32g..32g+5) in separate banks, rows
                    # outside stay zero.
                    nc.vector.tensor_mul(ht_st[:, i, :], uvs[:], p_v[:])
                    for g in range(NG):
                        nc.tensor.matmul(
                            p_h[32 * g : 32 * g + 16, :],
                            dwt_sb[:, i, g, :],
                            ht_st[:, i, :],
                            start=False,
                            stop=(i == N_BLOCKS - 1),
                            skip_group_check=True,
                            tile_position=(0, 32 * g),
                        )
                    hx = hp.tile([128, NT], dio, tag="hx", name="hx")
                    nc.vector.tensor_copy(hx[:], p_h[:])

                for g in range(NG):
                    c0 = s * SUP + g * NT
                    nc.scalar.dma_start(
                        hT_r[:, :, c0 : c0 + NT], ht_st[32 * g : 32 * g + 6, :, :]
                    )

                p_lg = plg[0]
                for g in range(NG):
                    nc.tensor.matmul(
                        p_lg[32 * g : 32 * g + 10, :],
                        wht_sb[:, g, :],
                        hx[:],
                        start=True,
                        stop=True,
                        tile_position=(0, 32 * g),
                    )
                lg_sb = lp.tile([128, NT], dio, tag="lg", name="lg")
                nc.vector.tensor_copy(lg_sb[:], p_lg[:])
                for g in range(NG):
                    c0 = s * SUP + g * NT
                    nc.sync.dma_start(
                        lgT[:, c0 : c0 + NT], lg_sb[32 * g : 32 * g + 10, :]
                    )

    nc.compile()
    return nc


def _prep_weights(W_embed, L_w, R_w, D_w, W_head):
    """Pack weights into partition-replicated, zero-padded lhsT layouts."""
    WeT = np.ascontiguousarray(W_embed.T)  # [784, 16]
    wet = np.zeros((128, 7, 16), np.float32)
    for q in range(NCHUNK_FULL):
        wet[:, q, :] = WeT[128 * q : 128 * (q + 1)]
    wet[:K_LAST, NCHUNK_FULL, :] = WeT[128 * NCHUNK_FULL :]

    lw = np.zeros((128, N_BLOCKS, NG, 6), np.float32)
    rw = np.zeros((128, N_BLOCKS, NG, 6), np.float32)
    dwt = np.zeros((128, N_BLOCKS, NG, 16), np.float32)
    wht = np.zeros((128, NG, 10), np.float32)
    for i in range(N_BLOCKS):
        for g in range(NG):
            lw[32 * g : 32 * g + 16, i, g, :] = L_w[i].T
            rw[32 * g : 32 * g + 16, i, g, :] = R_w[i].T
            dwt[32 * g : 32 * g + 6, i, g, :] = D_w[i].T
    for g in range(NG):
        wht[32 * g : 32 * g + 16, g, :] = W_head.T
    return wet, lw, rw, dwt, wht


MM_DT = os.environ.get("KERNEL_MM_DT", "f32")


def _get_nc():
    key = ("nc", MM_DT)
    if key not in _CACHE:
        _CACHE[key] = _build_nc(debug=False, mm_dt=MM_DT)
    return _CACHE[key]


def _run(inputs, trace=False):
    from concourse.bass_utils import run_bass_kernel_spmd

    x = np.asarray(inputs["x"], np.float32)
    wet, lw, rw, dwt, wht = _prep_weights(
        np.asarray(inputs["W_embed"], np.float32),
        np.asarray(inputs["L_w"], np.float32),
        np.asarray(inputs["R_w"], np.float32),
        np.asarray(inputs["D_w"], np.float32),
        np.asarray(inputs["W_head"], np.float32),
    )

    in_maps = []
    for c in range(N_CORES):
        xc = np.ascontiguousarray(x[c * B_CORE : (c + 1) * B_CORE].T)
        in_maps.append(
            {"xT": xc, "wet": wet, "lw": lw, "rw": rw, "dwt": dwt, "wht": wht}
        )

    nc = _get_nc()
    try:
        res = run_bass_kernel_spmd(
            nc, in_maps, core_ids=list(range(N_CORES)), trace=trace
        )
    except Exception:
        # transient device errors (e.g. NRT_EXEC_UNIT_UNRECOVERABLE) recover
        # on retry
        res = run_bass_kernel_spmd(
            nc, in_maps, core_ids=list(range(N_CORES)), trace=trace
        )

    logits = np.concatenate([np.asarray(r["lgT"]).T for r in res.results], axis=0)
    hs = []
    for i in range(N_BLOCKS):
        hs.append(
            np.concatenate([np.asarray(r["hT"])[i].T for r in res.results], axis=0)
        )
    outs = (np.ascontiguousarray(logits, dtype=np.float32),) + tuple(
        np.ascontiguousarray(h, dtype=np.float32) for h in hs
    )
    return outs, res


def kernel(**inputs):
    outs, _ = _run(inputs, trace=False)
    return outs


# revision 26
# speedup vs baseline: 1.0264x; 1.0264x over previous
"""Trainium2 Bass kernel for BilinearResNet (batch-data-parallel over 8 cores).

kernel(**inputs) takes the FULL unsharded numpy inputs (keys as in
setup_inputs()) and returns the full output tuple (logits, h1..h4), matching
reference() in shapes/dtypes. Full fp32 precision (rel err ~5e-7).

Design (per core, batch shard of 8192 rows):
  - Host pre-transposes x so the kernel streams xT [784, B] with perfectly
    coalesced DMA; all on-chip activations are kept transposed
    [feature, batch]; outputs are written transposed and un-transposed on
    the host. No on-chip transposes anywhere.
  - Every matmul contracts over the partition dim. 4 batch groups of 512
    columns run concurrently on the PE via 4x column tiling
    (tile_position=(0, 32g)), which also packs the 4 groups into one PSUM
    bank so each DVE/ACT op serves 2048 batch columns.
  - The residual stream h_x accumulates in PSUM across embed (7 K-chunks)
    plus the 4 per-block D-matmuls as one accumulation group per group
    strip; u and v go to separate PSUM banks at the same partitions so
    u*v is a single partition-aligned DVE multiply per block.
  - Weights are host-packed into zero-padded, partition-replicated lhsT
    layouts so every matmul is K<=128 with no fixups; the leftover
    K=16 embed chunk rides in a zeroed full-height tile.
  - Queues: x loads on Sync (HWDGE), h outputs on Scalar (HWDGE), logits
    on Sync, xlast on GpSimd (SWDGE) - chain-critical PSUM evacuation
    stays on DVE so DMA-lane stalls never back up the compute chain.
"""

import os
import sys

for _p in ("/opt/trn_rl_repo", "/root/.axon_site/_ro/trn_rl_repo"):
    if os.path.isdir(_p) and _p not in sys.path:
        sys.path.append(_p)

import numpy as np

# Problem constants (hardcoded per harness contract).
IN_DIM, D_MODEL, HIDDEN, N_BLOCKS, N_CLASSES = 784, 16, 6, 4, 10
BATCH = 65536
N_CORES = 8
B_CORE = BATCH // N_CORES  # 8192
NSUP = 4                   # super-tiles per core
SUP = B_CORE // NSUP       # 2048 batch columns per super-tile
NT = 512                   # batch columns per PE column-tile group (PSUM bank)
NG = 4                     # concurrent column-tile groups
NCHUNK_FULL = IN_DIM // 128          # 6 full K chunks
K_LAST = IN_DIM - 128 * NCHUNK_FULL  # 16 leftover contraction rows

_CACHE = {}


def _build_nc(debug=False, mm_dt="f32"):
    import concourse.bacc as bacc
    import concourse.mybir as mybir
    import concourse.tile as tile

    f32 = mybir.dt.float32
    # "f32r": declare the whole fp32 dataflow as float32r so the PE streams
    # it single-pass (1 cycle/row) instead of fp32's 4; PSUM stays fp32.
    dio = mybir.dt.float32r if mm_dt == "f32r" else f32
    nc = bacc.Bacc(
        "TRN2",
        target_bir_lowering=False,
        debug=debug,
        num_devices=N_CORES,
    )

    xT = nc.dram_tensor("xT", [IN_DIM, B_CORE], dio, kind="ExternalInput").ap()
    wet = nc.dram_tensor("wet", [128, 7, 16], dio, kind="ExternalInput").ap()
    lw = nc.dram_tensor("lw", [128, N_BLOCKS, NG, 6], dio, kind="ExternalInput").ap()
    rw = nc.dram_tensor("rw", [128, N_BLOCKS, NG, 6], dio, kind="ExternalInput").ap()
    dwt = nc.dram_tensor("dwt", [128, N_BLOCKS, NG, 16], dio, kind="ExternalInput").ap()
    wht = nc.dram_tensor("wht", [128, NG, 10], dio, kind="ExternalInput").ap()
    lgT = nc.dram_tensor("lgT", [N_CLASSES, B_CORE], dio, kind="ExternalOutput").ap()
    hT = nc.dram_tensor("hT", [N_BLOCKS, HIDDEN, B_CORE], dio, kind="ExternalOutput").ap()
    # [r, i, c] view so one DMA per group can write all 4 blocks' h rows.
    hT_r = hT.rearrange("i r c -> r i c")

    with tile.TileContext(nc) as tc:
        from contextlib import ExitStack

        with ExitStack() as stack:
            wp = stack.enter_context(tc.tile_pool(name="weights", bufs=1))
            xp = stack.enter_context(tc.tile_pool(name="xin", bufs=1))
            hp = stack.enter_context(tc.tile_pool(name="hx", bufs=4))
            up = stack.enter_context(tc.tile_pool(name="uv", bufs=3))
            tp = stack.enter_context(tc.tile_pool(name="ht", bufs=2))
            lp = stack.enter_context(tc.tile_pool(name="lg", bufs=2))
            pp = stack.enter_context(tc.tile_pool(name="ps", bufs=1, space="PSUM"))

            wet_sb = wp.tile([128, 7, 16], dio, tag="wet")
            lw_sb = wp.tile([128, N_BLOCKS, NG, 6], dio, tag="lw")
            rw_sb = wp.tile([128, N_BLOCKS, NG, 6], dio, tag="rw")
            dwt_sb = wp.tile([128, N_BLOCKS, NG, 16], dio, tag="dwt")
            wht_sb = wp.tile([128, NG, 10], dio, tag="wht")
            weight_loads = [
                (wet_sb, wet), (lw_sb, lw), (rw_sb, rw), (dwt_sb, dwt), (wht_sb, wht)
            ]

            # Leftover K chunk (rows 768:784 of xT) lives in partitions 0:16 of
            # a full-height tile; partitions 16:128 are zeroed once so the
            # K=128 embed matmul against zero-padded weights reads finite data.
            xlast = xp.tile([128, B_CORE], dio, tag="xlast")
            nc.gpsimd.memset(xlast[:].bitcast(f32), 0.0)
            nc.gpsimd.dma_start(xlast[0:16, :], xT[768:784, :])

            xts = [
                xp.tile([128, NCHUNK_FULL, SUP], dio, tag=f"xts{j}", name=f"xts{j}")
                for j in (0, 1)
            ]
            ph = [pp.tile([128, NT], f32, tag=f"ph{j}", name=f"ph{j}") for j in (0, 1)]
            pu = [pp.tile([128, NT], f32, tag=f"pu{j}", name=f"pu{j}") for j in (0, 1)]
            pv = [pp.tile([128, NT], f32, tag=f"pv{j}", name=f"pv{j}") for j in (0, 1)]
            plg = [
                pp.tile([128, NT], f32, tag=f"plg{j}", name=f"plg{j}") for j in (0, 1)
            ]
            # One-time zero of u/v/logit banks: rows never written by their
            # matmuls stay finite (and zero) for the full-tile reads.
            for t in (*pu, *pv, *plg):
                nc.vector.memset(t[:], 0.0)

            for s in range(NSUP):
                sl = s % 2
                x_sb = xts[sl]
                if s == 0:
                    nc.sync.dma_start(weight_loads[0][0][:], weight_loads[0][1][:])
                for q in range(NCHUNK_FULL):
                    nc.sync.dma_start(
                        x_sb[:, q, :],
                        xT[128 * q : 128 * (q + 1), s * SUP : (s + 1) * SUP],
                    )
                if s == 0:
                    for dst, srcw in weight_loads[1:]:
                        nc.sync.dma_start(dst[:], srcw[:])
                p_h = ph[sl]
                for q in range(7):
                    for g in range(NG):
                        if q < NCHUNK_FULL:
                            rhs = x_sb[:, q, g * NT : (g + 1) * NT]
                        else:
                            c0 = s * SUP + g * NT
                            rhs = xlast[:, c0 : c0 + NT]
                        nc.tensor.matmul(
                            p_h[32 * g : 32 * g + 16, :],
                            wet_sb[:, q, :],
                            rhs,
                            start=(q == 0),
                            stop=False,
                            skip_group_check=True,
                            tile_position=(0, 32 * g),
                        )
                hx = hp.tile([128, NT], dio, tag="hx", name="hx")
                nc.vector.tensor_copy(hx[:], p_h[:])

                # h outputs for all 4 blocks stage here, then leave in one
                # DMA per group.
                ht_st = tp.tile([128, N_BLOCKS, NT], dio, tag="ht", name="ht")
                for i in range(N_BLOCKS):
                    p_u = pu[sl]
                    p_v = pv[sl]
                    for g in range(NG):
                        nc.tensor.matmul(
                            p_u[32 * g : 32 * g + 6, :],
                            lw_sb[:, i, g, :],
                            hx[:],
                            start=True,
                            stop=True,
                            tile_position=(0, 32 * g),
                        )
                    for g in range(NG):
                        nc.tensor.matmul(
                            p_v[32 * g : 32 * g + 6, :],
                            rw_sb[:, i, g, :],
                            hx[:],
                            start=True,
                            stop=True,
                            tile_position=(0, 32 * g),
                        )
                    uvs = up.tile([128, NT], dio, tag="uvs", name="uvs")
                    nc.vector.tensor_copy(uvs[:], p_u[:])
                    # h = u * v for all 4 groups at once; u and v live at the
                    # same partitions (32g..32g+5) in separate banks, rows
                    # outside stay zero.
                    nc.vector.tensor_mul(ht_st[:, i, :], uvs[:], p_v[:])
                    for g in range(NG):
                        nc.tensor.matmul(
                            p_h[32 * g : 32 * g + 16, :],
                            dwt_sb[:, i, g, :],
                            ht_st[:, i, :],
                            start=False,
                            stop=(i == N_BLOCKS - 1),
                            skip_group_check=True,
                            tile_position=(0, 32 * g),
                        )
                    hx = hp.tile([128, NT], dio, tag="hx", name="hx")
                    nc.vector.tensor_copy(hx[:], p_h[:])

                for g in range(NG):
                    c0 = s * SUP + g * NT
                    nc.scalar.dma_start(
                        hT_r[:, :, c0 : c0 + NT], ht_st[32 * g : 32 * g + 6, :, :]
                    )

                p_lg = plg[sl]
                for g in range(NG):
                    nc.tensor.matmul(
                        p_lg[32 * g : 32 * g + 10, :],
                        wht_sb[:, g, :],
                        hx[:],
                        start=True,
                        stop=True,
                        tile_position=(0, 32 * g),
                    )
                lg_sb = lp.tile([128, NT], dio, tag="lg", name="lg")
                nc.vector.tensor_copy(lg_sb[:], p_lg[:])
                for g in range(NG):
                    c0 = s * SUP + g * NT
                    nc.sync.dma_start(
                        lgT[:, c0 : c0 + NT], lg_sb[32 * g : 32 * g + 10, :]
                    )

    nc.compile()
    return nc


def _prep_weights(W_embed, L_w, R_w, D_w, W_head):
    """Pack weights into partition-replicated, zero-padded lhsT layouts."""
    WeT = np.ascontiguousarray(W_embed.T)  # [784, 16]
    wet = np.zeros((128, 7, 16), np.float32)
    for q in range(NCHUNK_FULL):
        wet[:, q, :] = WeT[128 * q : 128 * (q + 1)]
    wet[:K_LAST, NCHUNK_FULL, :] = WeT[128 * NCHUNK_FULL :]

    lw = np.zeros((128, N_BLOCKS, NG, 6), np.float32)
    rw = np.zeros((128, N_BLOCKS, NG, 6), np.float32)
    dwt = np.zeros((128, N_BLOCKS, NG, 16), np.float32)
    wht = np.zeros((128, NG, 10), np.float32)
    for i in range(N_BLOCKS):
        for g in range(NG):
            lw[32 * g : 32 * g + 16, i, g, :] = L_w[i].T
            rw[32 * g : 32 * g + 16, i, g, :] = R_w[i].T
            dwt[32 * g : 32 * g + 6, i, g, :] = D_w[i].T
    for g in range(NG):
        wht[32 * g : 32 * g + 16, g, :] = W_head.T
    return wet, lw, rw, dwt, wht


MM_DT = os.environ.get("KERNEL_MM_DT", "f32")


def _get_nc():
    key = ("nc", MM_DT)
    if key not in _CACHE:
        _CACHE[key] = _build_nc(debug=False, mm_dt=MM_DT)
    return _CACHE[key]


def _run(inputs, trace=False):
    from concourse.bass_utils import run_bass_kernel_spmd

    x = np.asarray(inputs["x"], np.float32)
    wet, lw, rw, dwt, wht = _prep_weights(
        np.asarray(inputs["W_embed"], np.float32),
        np.asarray(inputs["L_w"], np.float32),
        np.asarray(inputs["R_w"], np.float32),
        np.asarray(inputs["D_w"], np.float32),
        np.asarray(inputs["W_head"], np.float32),
    )

    in_maps = []
    for c in range(N_CORES):
        xc = np.ascontiguousarray(x[c * B_CORE : (c + 1) * B_CORE].T)
        in_maps.append(
            {"xT": xc, "wet": wet, "lw": lw, "rw": rw, "dwt": dwt, "wht": wht}
        )

    nc = _get_nc()
    try:
        res = run_bass_kernel_spmd(
            nc, in_maps, core_ids=list(range(N_CORES)), trace=trace
        )
    except Exception:
        # transient device errors (e.g. NRT_EXEC_UNIT_UNRECOVERABLE) recover
        # on retry
        res = run_bass_kernel_spmd(
            nc, in_maps, core_ids=list(range(N_CORES)), trace=trace
        )

    logits = np.concatenate([np.asarray(r["lgT"]).T for r in res.results], axis=0)
    hs = []
    for i in range(N_BLOCKS):
        hs.append(
            np.concatenate([np.asarray(r["hT"])[i].T for r in res.results], axis=0)
        )
    outs = (np.ascontiguousarray(logits, dtype=np.float32),) + tuple(
        np.ascontiguousarray(h, dtype=np.float32) for h in hs
    )
    return outs, res


def kernel(**inputs):
    outs, _ = _run(inputs, trace=False)
    return outs
